# revision 8
# baseline (speedup 1.0000x reference)
"""LRU (diagonal complex linear recurrence) Trainium2 Bass kernel.

Math (per batch b, channel h, time t = 0..L-1):
    u_t   = delta * (x_t @ B_real + i * x_t @ B_img)        # input projection
    h_t   = lam * h_{t-1} + u_t,   h_{-1} = h0,  lam = r e^{i theta}
    out_t = Re(h_t)

Polar trick: h_t = e^{i theta (t+1)} g_t with g_t = r g_{t-1} + e^{-i theta (t+1)} u_t,
g_{-1} = h0. r is REAL so Re(g)/Im(g) decouple into two real first-order scans
-> native DVE tensor_tensor_scan. Rotation tables cos/sin(theta*(t+1)) are
data-independent -> precomputed host-side in float64.

Sharding: batch-parallel over 8 cores (2 batch elements per core), SPMD single
program via run_bass_kernel_spmd.

On-chip layout: (partitions = 128 channels per h-group, free = time).
x is PE-transposed (fp32) so the GEMM moving operand has f on partitions.
"""

import os
from contextlib import ExitStack

import numpy as np

import concourse.bass as bass
import concourse.tile as tile
from concourse import bacc, mybir
from concourse.masks import make_identity

B, L, F, H = 16, 4096, 512, 512
N_CORES = 8
B_LOC = B // N_CORES
HG = H // 128                 # h-groups (partition blocks of channels)
FG = F // 128                 # f-groups (contraction tiles)
TC = 512                      # time chunk (= one PSUM bank of fp32)
NTC = L // TC
FP32 = mybir.dt.float32

GEMM_DTYPE = os.environ.get("LRU_GEMM_DTYPE", "fp32r")  # "fp32" | "fp32r"
ABLATE = set(os.environ.get("LRU_ABLATE", "").split(","))
MM_DT = mybir.dt.float32r if GEMM_DTYPE == "fp32r" else mybir.dt.float32
A = mybir.AluOpType


def build_program():
    nc = bacc.Bacc("TRN2", target_bir_lowering=False, debug=False,
                   enable_asserts=False, num_devices=1)

    x_d = nc.dram_tensor("x", [B_LOC, L, F], FP32, kind="ExternalInput").ap()
    br_d = nc.dram_tensor("btr", [F, H], FP32, kind="ExternalInput").ap()
    bi_d = nc.dram_tensor("bti", [F, H], FP32, kind="ExternalInput").ap()
    r_d = nc.dram_tensor("rvec", [H], FP32, kind="ExternalInput").ap()
    cos_d = nc.dram_tensor("ctab", [H, L], FP32, kind="ExternalInput").ap()
    sin_d = nc.dram_tensor("stab", [H, L], FP32, kind="ExternalInput").ap()
    h0r_d = nc.dram_tensor("h0r", [H], FP32, kind="ExternalInput").ap()
    h0i_d = nc.dram_tensor("h0i", [H], FP32, kind="ExternalInput").ap()
    out_d = nc.dram_tensor("out", [B_LOC, L, H], FP32, kind="ExternalOutput").ap()

    with tile.TileContext(nc) as tc, ExitStack() as ctx:
        singles = ctx.enter_context(tc.tile_pool(name="singles", bufs=1))
        xt_pool = ctx.enter_context(tc.tile_pool(name="xt", bufs=1))
        nat_pool = ctx.enter_context(tc.tile_pool(name="nat", bufs=2))
        tab_pool = ctx.enter_context(tc.tile_pool(name="tabs", bufs=3))
        work = ctx.enter_context(tc.tile_pool(name="work", bufs=2))
        ps_tr = ctx.enter_context(tc.tile_pool(name="ps_tr", bufs=2, space="PSUM"))
        ps_mm = ctx.enter_context(tc.tile_pool(name="ps_mm", bufs=2, space="PSUM"))
        ps_out = ctx.enter_context(tc.tile_pool(name="ps_out", bufs=2, space="PSUM"))

        ident = singles.tile([128, 128], FP32)
        make_identity(nc, ident)

        # B~ weights resident: (128 f-part, FG, H) so [:, fg, hg*128:...] is a
        # (128x128) stationary tile.
        btr_s = singles.tile([128, FG, H], MM_DT)
        bti_s = singles.tile([128, FG, H], MM_DT)
        btmp_r = singles.tile([128, FG, H], FP32)
        btmp_i = singles.tile([128, FG, H], FP32)
        nc.sync.dma_start(out=btmp_r, in_=br_d.rearrange("(fg p) h -> p fg h", p=128))
        nc.sync.dma_start(out=btmp_i, in_=bi_d.rearrange("(fg p) h -> p fg h", p=128))
        nc.vector.tensor_copy(btr_s, btmp_r)
        nc.vector.tensor_copy(bti_s, btmp_i)

        # channel params
        r_s = singles.tile([128, HG], FP32)
        h0r_s = singles.tile([128, HG], FP32)
        h0i_s = singles.tile([128, HG], FP32)
        nc.sync.dma_start(out=r_s, in_=r_d.rearrange("(hg p) -> p hg", p=128))
        nc.sync.dma_start(out=h0r_s, in_=h0r_d.rearrange("(hg p) -> p hg", p=128))
        nc.sync.dma_start(out=h0i_s, in_=h0i_d.rearrange("(hg p) -> p hg", p=128))
        ones = singles.tile([128, TC], FP32)
        nc.vector.memset(ones, 1.0)
        # r broadcast along time for scan data0, one per h-group
        r_bc = singles.tile([128, HG, TC], FP32)
        for hg in range(HG):
            nc.vector.tensor_scalar(r_bc[:, hg, :], ones, r_s[:, hg:hg + 1],
                                    None, op0=A.mult)

        for b in range(B_LOC):
            # --- transpose x[b] -> xt[fg] (128 f-part, L t-free) ---
            xt = [xt_pool.tile([128, L], MM_DT, tag=f"xt{fg}", name=f"xt{fg}_{b}")
                  for fg in range(FG)]
            for tcn in range(NTC):
                nat = [nat_pool.tile([128, F], FP32, tag=f"nat{sb}", name=f"nat{sb}_{b}_{tcn}")
                       for sb in range(TC // 128)]
                for sb in range(TC // 128):
                    t0 = tcn * TC + sb * 128
                    nc.sync.dma_start(out=nat[sb], in_=x_d[b, t0:t0 + 128, :])
                for fg in range(FG):
                    pt = ps_tr.tile([128, TC], FP32, tag="pt")
                    for sb in range(TC // 128):
                        nc.tensor.transpose(
                            pt[:, sb * 128:(sb + 1) * 128],
                            nat[sb][:, fg * 128:(fg + 1) * 128], ident)
                    nc.scalar.copy(out=xt[fg][:, tcn * TC:(tcn + 1) * TC], in_=pt)

            for hg in range(HG):
                hsl = slice(hg * 128, (hg + 1) * 128)
                for tcn in range(NTC):
                    sl = slice(tcn * TC, (tcn + 1) * TC)
                    ct = tab_pool.tile([128, TC], FP32, tag="ct")
                    st = tab_pool.tile([128, TC], FP32, tag="st")
                    if "tab" not in ABLATE:
                        nc.sync.dma_start(out=ct, in_=cos_d[hsl, sl])
                        nc.sync.dma_start(out=st, in_=sin_d[hsl, sl])

                    pur = ps_mm.tile([128, TC], FP32, tag="pur")
                    pui = ps_mm.tile([128, TC], FP32, tag="pui")
                    if "gemm" in ABLATE:
                        nc.vector.memset(pur, 0.0)
                        nc.vector.memset(pui, 0.0)
                    else:
                     for fg in range(FG):
                        nc.tensor.matmul(pur, btr_s[:, fg, hsl], xt[fg][:, sl],
                                         start=(fg == 0), stop=(fg == FG - 1))
                     for fg in range(FG):
                        nc.tensor.matmul(pui, bti_s[:, fg, hsl], xt[fg][:, sl],
                                         start=(fg == 0), stop=(fg == FG - 1))

                    # rotate into scan inputs: vr = c*ur + s*ui, vi = c*ui - s*ur
                    t1 = work.tile([128, TC], FP32, tag="t1")
                    t2 = work.tile([128, TC], FP32, tag="t2")
                    t3 = work.tile([128, TC], FP32, tag="t3")
                    t4 = work.tile([128, TC], FP32, tag="t4")
                    vr = work.tile([128, TC], FP32, tag="vr")
                    vi = work.tile([128, TC], FP32, tag="vi")
                    if "rot" in ABLATE:
                        nc.vector.tensor_copy(vr, pur)
                        nc.vector.tensor_copy(vi, pui)
                    else:
                        nc.vector.tensor_mul(t1, ct, pur)
                        nc.vector.tensor_mul(t2, st, pui)
                        nc.vector.tensor_mul(t3, ct, pui)
                        nc.vector.tensor_mul(t4, st, pur)
                        nc.vector.tensor_add(vr, t1, t2)
                        nc.vector.tensor_sub(vi, t3, t4)

                    # chained scans
                    gr = work.tile([128, TC], FP32, tag="gr")
                    gi = work.tile([128, TC], FP32, tag="gi")
                    if tcn == 0:
                        init_r, init_i = h0r_s[:, hg:hg + 1], h0i_s[:, hg:hg + 1]
                    else:
                        init_r, init_i = gr_prev[:, TC - 1:TC], gi_prev[:, TC - 1:TC]
                    if "scan" in ABLATE:
                        nc.vector.tensor_copy(gr, vr)
                        nc.vector.tensor_copy(gi, vi)
                    else:
                        nc.vector.tensor_tensor_scan(gr, r_bc[:, hg, :], vr, init_r,
                                                     op0=A.mult, op1=A.add)
                        nc.vector.tensor_tensor_scan(gi, r_bc[:, hg, :], vi, init_i,
                                                     op0=A.mult, op1=A.add)
                    gr_prev, gi_prev = gr, gi

                    # out = c*gr - s*gi  (o2/res on GPSIMD to offload DVE)
                    o1 = work.tile([128, TC], FP32, tag="o1")
                    o2 = work.tile([128, TC], FP32, tag="o2")
                    res = work.tile([128, TC], FP32, tag="res")
                    if "orot" in ABLATE:
                        nc.vector.tensor_copy(res, gr)
                    else:
                        nc.vector.tensor_mul(o1, ct, gr)
                        nc.gpsimd.tensor_tensor(o2, st, gi, op=A.mult)
                        nc.gpsimd.tensor_tensor(res, o1, o2, op=A.subtract)

                    if "out" not in ABLATE:
                        # transpose res (h,t)->(t,h) via PE, DMA from PSUM in
                        # natural DRAM layout (contiguous 128ch*4B runs)
                        pres = ps_out.tile([128, TC], FP32, tag="pres")
                        for sb in range(TC // 128):
                            nc.tensor.transpose(
                                pres[:, sb * 128:(sb + 1) * 128],
                                res[:, sb * 128:(sb + 1) * 128], ident)
                        rest = work.tile([128, TC], FP32, tag="rest")
                        nc.scalar.copy(out=rest, in_=pres)
                        nc.sync.dma_start(
                            out=out_d[b, sl, hsl].rearrange("(sb p) h -> p sb h", p=128),
                            in_=rest)

    nc.compile()
    return nc


def _prepare(inputs):
    x = np.asarray(inputs["x"], dtype=np.float32)
    B_real = np.asarray(inputs["B_real"], dtype=np.float32)
    B_img = np.asarray(inputs["B_img"], dtype=np.float32)
    nu = np.asarray(inputs["nu"], dtype=np.float64)
    theta = np.asarray(inputs["theta"], dtype=np.float64)
    delta = np.asarray(inputs["delta"], dtype=np.float32)
    h0r = np.asarray(inputs["h0_real"], dtype=np.float32)
    h0i = np.asarray(inputs["h0_img"], dtype=np.float32)

    btr = np.ascontiguousarray(B_real * delta[None, :], dtype=np.float32)
    bti = np.ascontiguousarray(B_img * delta[None, :], dtype=np.float32)
    r = np.exp(-np.exp(nu)).astype(np.float32)
    ang = theta[:, None] * np.arange(1, L + 1, dtype=np.float64)[None, :]
    ctab = np.cos(ang).astype(np.float32)
    stab = np.sin(ang).astype(np.float32)
    return x, btr, bti, r, ctab, stab, h0r, h0i


_NC_CACHE = {}


def get_program():
    key = GEMM_DTYPE
    if key not in _NC_CACHE:
        _NC_CACHE[key] = build_program()
    return _NC_CACHE[key]


def make_in_maps(inputs):
    x, btr, bti, r, ctab, stab, h0r, h0i = _prepare(inputs)
    shared = dict(btr=btr, bti=bti, rvec=r, ctab=ctab, stab=stab,
                  h0r=h0r, h0i=h0i)
    return [dict(x=np.ascontiguousarray(x[c * B_LOC:(c + 1) * B_LOC]), **shared)
            for c in range(N_CORES)]


def kernel(**inputs) -> np.ndarray:
    from concourse.bass_utils import run_bass_kernel_spmd

    nc = get_program()
    in_maps = make_in_maps(inputs)
    res = run_bass_kernel_spmd(nc, in_maps, list(range(N_CORES)))
    out = np.empty((B, L, H), dtype=np.float32)
    for c in range(N_CORES):
        out[c * B_LOC:(c + 1) * B_LOC] = res.results[c]["out"]
    return out
